# revision 34
# baseline (speedup 1.0000x reference)
"""MoE layer (8 experts, top-2, capacity 2560) on 8 Trainium2 NeuronCores.

Expert-parallel: one expert per core. Host does gating/routing (the
data-dependent "sharding"/dispatch step) and the final weighted combine;
each core runs the dense expert FFN  relu(buf @ w1 + b1) @ w2 + b2  for
its expert over the filled part of the capacity-padded dispatch buffer.

Device kernel (per core): the expert's filled rows are processed in
chunks of CHUNK tokens (count chosen at runtime from the max expert
load, so padded all-zero capacity rows are not computed). Chunks are
processed in groups of 2 (or a trailing group of 1) so each streamed
weight tile serves the whole group. Per chunk:
  layer 1: for each of 64 hidden tiles (128 rows of H), accumulate 16
    matmuls (contraction D=2048 in 128-tiles) into one PSUM bank, then
    ACT relu+bias into an SBUF-resident hidden tile [128, CHUNK] (bf16).
  layer 2: 8 half-sweeps of 2 output d-tiles; each d-tile accumulates
    64 matmuls (contraction H=8192), then ACT copy+bias to SBUF + DMA.
Weights stream from HBM (re-read once per group); activations stay in
SBUF. bf16 multiplies, fp32 PSUM accumulation.

DMA issue costs ~0.6us per dma_start on an HWDGE queue, so transfers are
spread across both queues (sync + scalar) to keep either engine's issue
rate from gating the PE (critical for the trailing single-chunk group,
whose L2 consumes one w2 tile per ~0.7us). Startup latency is hidden two
ways: the first group's token buffers are DMA'd as 4-d-tile slices (so
the first matmul waits on ~0.5MB, not ~1.8MB), and a burst of dummy
matmuls on a zeroed tile keeps the PE busy from t~0 so the HAM clock
gate reaches full rate before the real stream.
"""

import math

import numpy as np
import ml_dtypes

import concourse.bacc as bacc
import concourse.mybir as mybir
import concourse.tile as tile
from concourse import bass_utils

F32 = mybir.dt.float32
BF16 = mybir.dt.bfloat16
AF = mybir.ActivationFunctionType

# Problem constants (from the reference module).
NUM_EXPERTS = 8
TOP_K = 2
D = 2048          # d_model
H = 8192          # d_hidden
B, S = 4, 2048
T = B * S         # 8192 tokens
CAP = 2560        # ceil(T*K/E * 1.25)

DT = 16           # d tiles (DT*128 == D)
HT = 64           # h tiles (HT*128 == H)

N_WARMUP_MM = 170  # dummy matmuls to warm the PE clock gate at startup

_CACHE = {}


def _build_nc(nchunk, chunk, last_n):
    # pairs first, single chunk (if any) last: a leading single makes its
    # bandwidth-hungry L2 collide with the next pair's buffer prefetch
    # (measured ~9us slower), and a trailing pair drains a longer tail
    groups = [2] * (nchunk // 2) + [1] * (nchunk % 2)
    # per-chunk matmul widths: the last chunk only computes the columns
    # that hold real tokens (saves (chunk-last_n)*2048 PE cycles)
    ns = [chunk] * (nchunk - 1) + [last_n]
    nc = bacc.Bacc("TRN2", target_bir_lowering=False, debug=False)
    bufx = nc.dram_tensor("bufx", [nchunk, 128, DT, chunk], BF16, kind="ExternalInput")
    w1x = nc.dram_tensor("w1x", [HT, 128, DT, 128], BF16, kind="ExternalInput")
    w2x = nc.dram_tensor("w2x", [8, HT // 2, 128, 2, 2, 128], BF16, kind="ExternalInput")
    b1x = nc.dram_tensor("b1x", [128, HT], F32, kind="ExternalInput")
    b2x = nc.dram_tensor("b2x", [128, DT], F32, kind="ExternalInput")
    outx = nc.dram_tensor("outx", [nchunk, DT, 128, chunk], F32, kind="ExternalOutput")

    with tile.TileContext(nc) as tc:
        with (
            tc.tile_pool(name="consts", bufs=1) as consts,
            tc.tile_pool(name="buf0p", bufs=8) as buf0p,
            tc.tile_pool(name="bufp", bufs=2) as bufp,
            tc.tile_pool(name="w1p", bufs=4) as w1p,
            tc.tile_pool(name="w2p", bufs=12) as w2p,
            tc.tile_pool(name="hp", bufs=2) as hp,
            tc.tile_pool(name="outp", bufs=4) as outp,
            tc.tile_pool(name="ps1", bufs=3, space="PSUM") as ps1,
            tc.tile_pool(name="ps2", bufs=5, space="PSUM") as ps2,
        ):
            # PE warmup: zero a small tile and issue dummy matmuls so the
            # HAM clock gate un-throttles while the first input DMAs land.
            wsrc = consts.tile([128, 64], BF16)
            nc.vector.memset(wsrc[:], 0)
            wps = ps1.tile([64, 64], F32, name="warm_ps", tag="ps1")
            for _ in range(N_WARMUP_MM):
                nc.tensor.matmul(wps[:], wsrc[:], wsrc[:], start=True, stop=True)

            # DMA issue is ~0.6us per dma_start on an HWDGE queue; spread
            # traffic over both queues (sync + scalar) so neither engine's
            # issue rate gates the PE: w1/w2 stream on sync (w2 alternating
            # with scalar), token buffers / biases / outputs on scalar.
            b1_sb = consts.tile([128, HT], F32)
            b2_sb = consts.tile([128, DT], F32)

            NSL = 4                      # startup slices per chunk (4 d-tiles)
            base = 0
            for gi, gsize in enumerate(groups):
                cids = list(range(base, base + gsize))
                base += gsize

                # ---- token buffers for this group ----
                if gi == 0:
                    # sliced tiles, interleaved issue: the first matmuls only
                    # wait for slice 0 of each chunk plus the first w1 tile
                    bufs = [
                        [buf0p.tile([128, DT // NSL, chunk], BF16,
                                    name=f"b0_{ci}_{s}", tag="b0")
                         for s in range(NSL)]
                        for ci in cids
                    ]
                    # split slice issue across BOTH queues: 10 serial issues
                    # on scalar alone delay the last slice to ~18us and the
                    # PE ladder catches up with it; chunk B's later slices
                    # ride the near-idle sync queue instead
                    for s in range(NSL):
                        for j, ci in enumerate(cids):
                            # A3 rides sync too: it was the last-landing slice
                            # and set the 3.7us ladder gap just over the 3.4us
                            # HAM window; landing it earlier avoids the dip
                            eng = nc.sync if (j == 1 and s > 0) or \
                                (j == 0 and s == NSL - 1) else nc.scalar
                            eng.dma_start(
                                bufs[j][s][:],
                                bufx[ci][:, s * (DT // NSL):(s + 1) * (DT // NSL), :])
                    # b1 is first needed by the ht0 relu (~18us in)
                    nc.scalar.dma_start(b1_sb[:], b1x[:])
                    nc.scalar.dma_start(b2_sb[:], b2x[:])
                    bview = [
                        [row[dt // (DT // NSL)][:, dt % (DT // NSL), :]
                         for dt in range(DT)]
                        for row in bufs
                    ]
                else:
                    bufs = [
                        bufp.tile([128, DT, chunk], BF16, name=f"buf_{ci}", tag="buf")
                        for ci in cids
                    ]
                    for j, ci in enumerate(cids):
                        nc.scalar.dma_start(bufs[j][:], bufx[ci])
                    bview = [[t[:, dt, :] for dt in range(DT)] for t in bufs]

                hTs = [
                    hp.tile([128, HT, chunk], BF16, name=f"hT_{ci}", tag="hT")
                    for ci in cids
                ]

                # ---- layer 1: hT[ht] = relu(w1[:,ht]^T @ bufT + b1[ht]) ----
                for ht in range(HT):
                    if gi == 0 and ht == 0:
                        # split the very first w1 tile so matmul (ht0, dt0)
                        # only waits for a quarter of it
                        w1_parts = [
                            w1p.tile([128, DT // NSL, 128], BF16,
                                     name=f"w1s_{s}", tag="w1s")
                            for s in range(NSL)
                        ]
                        for s in range(NSL):
                            nc.sync.dma_start(
                                w1_parts[s][:],
                                w1x[ht][:, s * (DT // NSL):(s + 1) * (DT // NSL), :])
                        w1view = [
                            w1_parts[dt // (DT // NSL)][:, dt % (DT // NSL), :]
                            for dt in range(DT)
                        ]
                    else:
                        w1_sb = w1p.tile([128, DT, 128], BF16)
                        nc.sync.dma_start(w1_sb[:], w1x[ht])
                        w1view = [w1_sb[:, dt, :] for dt in range(DT)]
                    pss = [
                        ps1.tile([128, chunk], F32, name=f"ps_{gi}_{ht}_{j}", tag="ps1")
                        for j in range(gsize)
                    ]
                    for dt in range(DT):
                        for j in range(gsize):
                            n = ns[cids[j]]
                            nc.tensor.matmul(
                                pss[j][:, :n], w1view[dt], bview[j][dt][:, :n],
                                start=(dt == 0), stop=(dt == DT - 1),
                            )
                    for j in range(gsize):
                        n = ns[cids[j]]
                        nc.scalar.activation(
                            hTs[j][:, ht, :n], pss[j][:, :n], AF.Relu,
                            bias=b1_sb[:, ht:ht + 1])


                # ---- layer 2: out[dt] = sum_ht w2[ht,dt]^T @ hT[ht] + b2 ----
                # 8 half-sweeps of 2 d-tiles x gsize chunks
                for dh in range(8):
                    pss = [
                        ps2.tile([128, chunk], F32, name=f"pso_{gi}_{dh}_{i}",
                                 tag="pso")
                        for i in range(2 * gsize)
                    ]
                    for hpi in range(HT // 2):
                        w2_sb = w2p.tile([128, 2, 2, 128], BF16)
                        dma_eng = nc.sync if hpi % 2 == 0 else nc.scalar
                        dma_eng.dma_start(w2_sb[:], w2x[dh, hpi])
                        for t in range(2):
                            ht = 2 * hpi + t
                            for i in range(2):
                                for j in range(gsize):
                                    n = ns[cids[j]]
                                    nc.tensor.matmul(
                                        pss[2 * j + i][:, :n], w2_sb[:, t, i, :],
                                        hTs[j][:, ht, :n],
                                        start=(ht == 0), stop=(ht == HT - 1),
                                    )
                    for i in range(2):
                        dt = dh * 2 + i
                        for j, cc in enumerate(cids):
                            n = ns[cc]
                            o_sb = outp.tile([128, chunk], F32)
                            nc.scalar.activation(
                                o_sb[:, :n], pss[2 * j + i][:, :n], AF.Identity,
                                bias=b2_sb[:, dt:dt + 1])
                            # alternate store queues so consecutive stores
                            # issue in parallel (matters for the final drain)
                            out_eng = nc.sync if (2 * j + i) % 2 == 0 else nc.scalar
                            out_eng.dma_start(outx[cc, dt][:, :n], o_sb[:, :n])
    nc.compile()
    return nc


def _get_nc(nchunk, chunk, last_n):
    key = (nchunk, chunk, last_n)
    if key not in _CACHE:
        _CACHE[key] = _build_nc(nchunk, chunk, last_n)
    return _CACHE[key]


def _pick_shape(max_rows):
    """Pick (nchunk, chunk) with nchunk*chunk >= max_rows, chunk a
    multiple of 4 and <= 512, minimizing the modeled tensor-engine time
    nchunk * (chunk/2.4 + 2.5) (per-matmul issue model, warm PE)."""
    best = None
    for nchunk in range(4, 24):
        chunk = int(math.ceil(max_rows / nchunk / 4)) * 4
        if chunk > 512:
            continue
        chunk = max(chunk, 128)
        cost = nchunk * (chunk / 2.4 + 2.5)
        if best is None or cost < best[0] - 1e-9:
            best = (cost, nchunk, chunk)
    assert best is not None
    nchunk, chunk = best[1], best[2]
    last_n = max(4, chunk - (nchunk * chunk - max_rows))
    return nchunk, chunk, last_n


def _route(x_flat, gating_w):
    """Gating softmax + top-k, replicating the reference's jax ops (same
    backend) so routing decisions match bitwise. Falls back to float64
    numpy if jax is unavailable."""
    try:
        import jax
        import jax.numpy as jnp

        gates = jax.nn.softmax(jnp.asarray(x_flat) @ jnp.asarray(gating_w), axis=-1)
        topk_w, topk_idx = jax.lax.top_k(gates, TOP_K)
        norm_w = topk_w / (jnp.sum(topk_w, axis=-1, keepdims=True) + 1e-8)
        return (np.asarray(topk_idx, dtype=np.int64),
                np.asarray(norm_w, dtype=np.float32))
    except Exception:
        logits = x_flat.astype(np.float64) @ gating_w.astype(np.float64)
        m = logits.max(axis=-1, keepdims=True)
        e = np.exp(logits - m)
        gates = (e / e.sum(axis=-1, keepdims=True)).astype(np.float32)
        # top-k with ties broken toward lower index, descending order
        order = np.argsort(-gates, axis=-1, kind="stable")
        topk_idx = order[:, :TOP_K]
        topk_w = np.take_along_axis(gates, topk_idx, axis=-1)
        norm_w = topk_w / (topk_w.sum(axis=-1, keepdims=True) + 1e-8)
        return topk_idx.astype(np.int64), norm_w.astype(np.float32)


def kernel(x, gating_w, w1, b1, w2, b2, **run_kwargs):
    x = np.ascontiguousarray(np.asarray(x, dtype=np.float32))
    gating_w = np.asarray(gating_w, dtype=np.float32)
    w1 = np.asarray(w1, dtype=np.float32)
    b1 = np.asarray(b1, dtype=np.float32)
    w2 = np.asarray(w2, dtype=np.float32)
    b2 = np.asarray(b2, dtype=np.float32)

    x_flat = x.reshape(T, D)

    # ---- routing (host) ----
    topk_idx, norm_w = _route(x_flat, gating_w)
    flat_e = topk_idx.reshape(-1)                       # [T*K]
    flat_t = np.repeat(np.arange(T, dtype=np.int64), TOP_K)
    flat_w = norm_w.reshape(-1)

    onehot = (flat_e[:, None] == np.arange(NUM_EXPERTS)[None, :]).astype(np.int32)
    pos_all = np.cumsum(onehot, axis=0) - 1
    position = pos_all[np.arange(T * TOP_K), flat_e]
    valid = position < CAP

    # Only the filled rows of each expert's capacity buffer need compute.
    counts = np.bincount(flat_e[valid], minlength=NUM_EXPERTS)
    max_rows = int(min(max(int(counts.max()), 128), CAP))
    nchunk, chunk, last_n = _pick_shape(max_rows)
    nrows = nchunk * chunk                              # >= max filled row
    nrows_real = (nchunk - 1) * chunk + last_n          # columns computed

    # ---- dispatch (host side of the "all-to-all") ----
    buf = np.zeros((NUM_EXPERTS, nrows, D), dtype=np.float32)
    buf[flat_e[valid], position[valid]] = x_flat[flat_t[valid]]

    # ---- per-core input packing ----
    in_maps = []
    for e in range(NUM_EXPERTS):
        bufx = (buf[e].reshape(nchunk, chunk, DT, 128).transpose(0, 3, 2, 1)
                .astype(ml_dtypes.bfloat16))
        w1x = (w1[e].reshape(DT, 128, HT, 128).transpose(2, 1, 0, 3)
               .astype(ml_dtypes.bfloat16))
        w2x = (w2[e].reshape(HT // 2, 2, 128, 8, 2, 128)
               .transpose(3, 0, 2, 1, 4, 5)
               .astype(ml_dtypes.bfloat16))
        b1x = np.ascontiguousarray(b1[e].reshape(HT, 128).T)
        b2x = np.ascontiguousarray(b2[e].reshape(DT, 128).T)
        in_maps.append({
            "bufx": np.ascontiguousarray(bufx),
            "w1x": np.ascontiguousarray(w1x),
            "w2x": np.ascontiguousarray(w2x),
            "b1x": b1x, "b2x": b2x,
        })

    # ---- run expert FFNs on the 8 cores ----
    nc = _get_nc(nchunk, chunk, last_n)
    res = bass_utils.run_bass_kernel_spmd(
        nc, in_maps, core_ids=list(range(NUM_EXPERTS)), **run_kwargs)
    if run_kwargs.get("trace"):
        _CACHE["last_results"] = res

    out_all = np.empty((NUM_EXPERTS, nrows, D), dtype=np.float32)
    for e in range(NUM_EXPERTS):
        out_all[e] = (res.results[e]["outx"].transpose(0, 3, 1, 2)
                      .reshape(nrows, D))
    # columns past last_n in the final chunk are never computed; zero them
    # so a clamped gather of a dropped item can't pick up garbage
    out_all[:, nrows_real:, :] = 0.0

    # ---- combine (host side of the "all-to-all" + weighted scatter-add) ----
    pos_g = np.minimum(position, nrows - 1)             # clamped rows get weight 0
    gathered = out_all[flat_e, pos_g]                   # [T*K, D]
    w_eff = np.where(valid, flat_w, 0.0).astype(np.float32)
    out_flat = (gathered * w_eff[:, None]).reshape(T, TOP_K, D).sum(axis=1)
    return out_flat.reshape(B, S, D).astype(np.float32)
